# revision 20
# baseline (speedup 1.0000x reference)
"""Causal self-attention kernel for TRN2 (8 NeuronCores, Bass/Tile).

Problem: B=8, T=1024, C=768, H=12, HD=64.
  qkv = x @ W_attn + b_attn ; causal softmax attention ; y = att_out @ W_proj + b_proj

Sharding: pure data-parallel over batch — core b computes batch element b.

Per-core dataflow (all matmuls in fp32r = full-rate reduced-precision fp32):
  xT   [768,1024]  = PE-transpose of x                      (lhsT/rhs source)
  qkT  [1536,1024] = (W_qk)^T-style projection: qkT[c',t] = sum_c W[c,c'] xT[c,t]
  V    [1024,768]  : V[t,c'] = sum_c xT[c,t] W_v[c,c']      (per-head Vp tiles with
                     a leading ones column -> PV matmul also produces Z row)
  per head h, i-block (512 cols):
     ST[j,i] = kT^T q  (K=64, causal-trimmed)   -> exp(0.125*ST) on ScalarE -> fp32r
     tri-mask on diagonal 128x128 sub-block (multiplicative, post-exp)
     OT'[0,:] = Z, OT'[1:65,:] = unnormalized attention out (transposed), accumulated
     ATn[c,t] = OT'[1:65]/Z  (DVE mult by gpsimd-broadcast 1/Z)
  y[t,:] = ATn^T-contraction with W_proj + b_proj
"""

import numpy as np

import concourse.bass as bass
import concourse.mybir as mybir
import concourse.tile as tile
from concourse import bacc
from concourse.bass_utils import run_bass_kernel_spmd

F32 = mybir.dt.float32
F32R = mybir.dt.float32r
BF16 = mybir.dt.bfloat16
AF = mybir.ActivationFunctionType
ALU = mybir.AluOpType

T, C, H, HD = 1024, 768, 12, 64
NCORES = 8
CC = C // 128          # 6 contraction chunks
TP = T // 128          # 8 t-chunks of 128
TB = T // 512          # 2 t-blocks of 512
QKCP = 2 * C // 128    # 12 qkT partition tiles
SCALE = 1.0 / 8.0      # 1/sqrt(64)

_PROGRAM_CACHE = {}


def build_program():
    nc = bacc.Bacc("TRN2", target_bir_lowering=False, debug=False)

    x_d = nc.dram_tensor("x", [T, C], BF16, kind="ExternalInput").ap()
    wa_d = nc.dram_tensor("W_attn", [C, 3 * C], BF16, kind="ExternalInput").ap()
    ba_d = nc.dram_tensor("b_attn", [1, 3 * C], BF16, kind="ExternalInput").ap()
    wp_d = nc.dram_tensor("W_proj", [C, C], BF16, kind="ExternalInput").ap()
    bp_d = nc.dram_tensor("b_proj", [1, C], BF16, kind="ExternalInput").ap()
    y_d = nc.dram_tensor("y", [T, C], F32, kind="ExternalOutput").ap()

    with tile.TileContext(nc) as tc:
        _emit(nc, tc, x_d, wa_d, ba_d, wp_d, bp_d, y_d)
    nc.compile()
    return nc


def _emit(nc, tc, x_d, wa_d, ba_d, wp_d, bp_d, y_d):
    from contextlib import ExitStack

    ctx = ExitStack()
    with ctx:
        const_pool = ctx.enter_context(tc.tile_pool(name="consts", bufs=1))
        # ps_work holds the merged [128,1024] ST tiles (2 banks each);
        # ps_acc holds 1-bank accumulation tiles (qk/v/y/ot').
        ps_work = ctx.enter_context(tc.tile_pool(name="ps_work", bufs=2, space="PSUM"))
        ps_acc = ctx.enter_context(tc.tile_pool(name="ps_acc", bufs=2, space="PSUM"))

        # ---- constants -------------------------------------------------
        ident_f32 = const_pool.tile([128, 128], F32, name="ident_f32")
        nc.gpsimd.memset(ident_f32[:], 0.0)
        nc.gpsimd.affine_select(
            out=ident_f32[:], in_=ident_f32[:], compare_op=ALU.not_equal, fill=1.0,
            base=0, pattern=[[-1, 128]], channel_multiplier=1,
        )
        ident = const_pool.tile([128, 128], BF16, name="ident")
        nc.vector.tensor_copy(ident[:], ident_f32[:])
        # tri[j, i] = 1.0 if j <= i else 0.0   (keep lower-causal in [j,i] layout)
        tri_f32 = const_pool.tile([128, 128], F32, name="tri_f32")
        nc.gpsimd.memset(tri_f32[:], 1.0)
        nc.gpsimd.affine_select(
            out=tri_f32[:], in_=tri_f32[:], compare_op=ALU.is_ge, fill=0.0,
            base=0, pattern=[[1, 128]], channel_multiplier=-1,
        )
        tri = const_pool.tile([128, 128], BF16, name="tri")
        nc.vector.tensor_copy(tri[:], tri_f32[:])
        ones32 = const_pool.tile([128, 16], F32, name="ones32")
        nc.gpsimd.memset(ones32[:], 1.0)
        ones_row = const_pool.tile([1, 512], BF16, name="ones_row")
        nc.gpsimd.memset(ones_row[:], 1.0)

        # warm the exp table set early (hidden under input DMA)
        expwarm = const_pool.tile([1, 1], F32, name="expwarm")
        nc.scalar.activation(expwarm[:], ones_row[0:1, 0:1], AF.Exp)

        # ---- phase A: load x (needed first), build xT [768, 1024] ------
        phase_ctx = ExitStack()
        xt_pool = phase_ctx.enter_context(tc.tile_pool(name="xt", bufs=1, side="right"))
        xsb_pool = phase_ctx.enter_context(tc.tile_pool(name="xsb", bufs=1, side="right"))

        xT = []
        for cc in range(CC):
            t_ = xt_pool.tile([128, T], BF16, name=f"xT_{cc}", tag=f"xT{cc}")
            xT.append(t_)
        # all 8 x chunks resident (bufs=8): the DMAs issue back-to-back with
        # no slot-reuse waits, so the W DMAs behind them on the queue are
        # never head-of-line blocked.
        xsb = []
        for tp in range(TP):
            x_sb = xsb_pool.tile([128, C], BF16, name=f"x_sb_{tp}", tag=f"x_sb{tp}")
            nc.sync.dma_start(x_sb[:], x_d[tp * 128 : (tp + 1) * 128, :])
            xsb.append(x_sb)

        # W_attn: 3 DMAs per row-chunk, V columns first, then q then k
        w_pool = phase_ctx.enter_context(tc.tile_pool(name="w", bufs=1, side="right"))
        W = []
        for cc in range(CC):
            w_t = w_pool.tile([128, 3 * C], BF16, name=f"W_{cc}", tag=f"W{cc}")
            W.append(w_t)
        for part in (2, 0, 1):
            for cc in range(CC):
                nc.sync.dma_start(
                    W[cc][:, part * C : (part + 1) * C],
                    wa_d[cc * 128 : (cc + 1) * 128, part * C : (part + 1) * C],
                )

        # bias loads go behind the big DMAs (first needed ~40us in; the
        # strided ba_col DMA has many tiny descriptors)
        ba_sb = const_pool.tile([1, 3 * C], BF16, name="ba_sb")
        nc.sync.dma_start(ba_sb[:], ba_d[:, :])
        bp_sb = const_pool.tile([1, C], BF16, name="bp_sb")
        nc.sync.dma_start(bp_sb[:], bp_d[:, :])
        # column layout of b_attn qk-part for per-partition bias add:
        # ba_col[p, cp] = b_attn[cp*128 + p]  (strided DMA, one-time, 3KB)
        ba_col = const_pool.tile([128, QKCP], F32, name="ba_col")
        ba_colb = const_pool.tile([128, QKCP], BF16, name="ba_colb")
        nc.sync.dma_start(
            ba_colb[:],
            ba_d[:, 0 : QKCP * 128].rearrange("a (cp p) -> (a p) cp", p=128),
        )
        nc.vector.tensor_copy(ba_col[:], ba_colb[:])

        for tp in range(TP):
            for cc in range(CC):
                pt = ps_work.tile([128, 128], BF16, name=f"ps_xt_{tp}_{cc}", tag="ps")
                nc.tensor.transpose(pt[:], xsb[tp][:, cc * 128 : (cc + 1) * 128], ident[:])
                nc.vector.tensor_copy(xT[cc][:, tp * 128 : (tp + 1) * 128], pt[:])

        # ---- phase B: Vp then qkT (attention needs all Vp) -------------
        vp_pool = ctx.enter_context(tc.tile_pool(name="vp", bufs=1))
        Vp = []
        for tp in range(TP):
            t_ = vp_pool.tile([128, H * 65], BF16, name=f"Vp_{tp}", tag=f"Vp{tp}")
            Vp.append(t_)
            nc.vector.tensor_copy(
                t_.rearrange("p (h e) -> p h e", e=65)[:, :, 64:65],
                ones32[:, 0:H].rearrange("p (h e) -> p h e", e=1),
            )

        def v_chunk(tp):
            for vc in range(2):  # v cols [1536+384*vc : 1536+384*(vc+1)]
                pv = ps_acc.tile([128, 384], F32, name=f"ps_v_{vc}_{tp}", tag="acc")
                for cc in range(CC):
                    nc.tensor.matmul(
                        pv[:],
                        xT[cc][:, tp * 128 : (tp + 1) * 128],
                        W[cc][:, 1536 + vc * 384 : 1536 + (vc + 1) * 384],
                        start=(cc == 0),
                        stop=False,
                    )
                nc.tensor.matmul(
                    pv[:],
                    ones_row[:, 0:128],
                    ba_sb[:, 1536 + vc * 384 : 1536 + (vc + 1) * 384],
                    start=False,
                    stop=True,
                )
                for hh in range(6):  # heads 6*vc + hh
                    h = 6 * vc + hh
                    nc.scalar.copy(
                        Vp[tp][:, h * 65 : h * 65 + 64],
                        pv[:, hh * 64 : (hh + 1) * 64],
                    )

        qkt_pool = ctx.enter_context(tc.tile_pool(name="qkt", bufs=1))
        qkT = []
        for cp in range(QKCP):
            t_ = qkt_pool.tile([128, T], BF16, name=f"qkT_{cp}", tag=f"qkT{cp}")
            qkT.append(t_)

        def qk_pair(hp):
            for tb in range(TB):
                for cp in (hp, 6 + hp):
                    pq = ps_acc.tile([128, 512], F32, name=f"ps_qk_{cp}_{tb}", tag="acc")
                    for cc in range(CC):
                        nc.tensor.matmul(
                            pq[:],
                            W[cc][:, cp * 128 : (cp + 1) * 128],
                            xT[cc][:, tb * 512 : (tb + 1) * 512],
                            start=(cc == 0),
                            stop=(cc == CC - 1),
                        )
                    # b_attn[c'] folded in as a per-partition scalar add
                    nc.vector.tensor_scalar_add(
                        qkT[cp][:, tb * 512 : (tb + 1) * 512],
                        pq[:],
                        ba_col[:, cp : cp + 1],
                    )

        # ---- W_proj prefetch ------------------------------------------
        wp_pool = ctx.enter_context(tc.tile_pool(name="wp", bufs=1))
        Wp = []
        for cc in range(CC):
            w_t = wp_pool.tile([128, C], BF16, name=f"Wp_{cc}", tag=f"Wp{cc}")
            nc.sync.dma_start(w_t[:], wp_d[cc * 128 : (cc + 1) * 128, :])
            Wp.append(w_t)

        # ---- phase C/D: attention (ib-major) interleaved with proj -----
        atn_pool = ctx.enter_context(tc.tile_pool(name="atn", bufs=1))
        ATn = []
        for cp in range(CC):
            t_ = atn_pool.tile([128, T], BF16, name=f"ATn_{cp}", tag=f"ATn{cp}")
            ATn.append(t_)

        est_pool = ctx.enter_context(tc.tile_pool(name="est", bufs=8))
        nrm_pool = ctx.enter_context(tc.tile_pool(name="nrm", bufs=3))
        y_pool = ctx.enter_context(tc.tile_pool(name="ysb", bufs=2))

        def attention(hp, ib):
            qt = qkT[hp]
            kt = qkT[6 + hp]
            po = {}
            for s in range(2):  # head 2*hp + s
                po[s] = ps_acc.tile([65, 512], F32, name=f"ps_ot_{hp}_{ib}_{s}", tag="ot", bufs=2)
            njc = 4 * (ib + 1)
            for jc in range(njc):
                r = jc - 4 * ib
                col0 = max(r, 0) * 128
                # merged pair tile: head A in cols [0:512], head B in [512:1024]
                pst = ps_work.tile([128, 1024], F32, name=f"ps_st_{hp}_{ib}_{jc}", tag="ps")
                for s in range(2):
                    r0 = 64 * s
                    # row-packed pair: s=0 uses PE rows 0-63, s=1 rows 64-127
                    nc.tensor.matmul(
                        pst[:, 512 * s + col0 : 512 * s + 512],
                        kt[r0 : r0 + 64, jc * 128 : (jc + 1) * 128],
                        qt[r0 : r0 + 64, ib * 512 + col0 : (ib + 1) * 512],
                        start=True,
                        stop=True,
                    )
                est = est_pool.tile([128, 1024], BF16, name=f"est_{hp}_{ib}_{jc}", tag="est")
                nw = 512 - col0
                nc.scalar.activation(
                    est.rearrange("p (a f) -> p a f", a=2)[:, :, col0:512],
                    pst.rearrange("p (a f) -> p a f", a=2)[:, :, col0:512],
                    AF.Exp,
                    scale=SCALE,
                )
                if r >= 0:
                    for s in range(2):
                        # mask the diagonal 128x128 sub-block (multiplicative)
                        nc.vector.tensor_tensor(
                            est[:, 512 * s + col0 : 512 * s + col0 + 128],
                            est[:, 512 * s + col0 : 512 * s + col0 + 128],
                            tri[:],
                            op=ALU.mult,
                        )
                for s in range(2):
                    h = 2 * hp + s
                    nc.tensor.matmul(
                        po[s][:, col0:512],
                        Vp[jc][:, h * 65 : h * 65 + 65],
                        est[:, 512 * s + col0 : 512 * s + 512],
                        start=(jc == 0),
                        stop=(jc == njc - 1),
                    )
            # normalization: ATn rows = OT'[0:64] / Z  (Z = row 64).
            # Copy OT' to SBUF right away so the PSUM slot frees in ~0.7us;
            # the (long-latency, off-critical-path) normalization then runs
            # entirely from SBUF. [1,512] DVE reciprocal is ~3.4us (single
            # partition, 8 cyc/elem): scatter Z across 128 partitions via
            # DMA, reciprocal at [128,4], gather back.
            for s in range(2):
                otu = nrm_pool.tile([65, 512], F32, name=f"otu_{hp}_{ib}_{s}", tag="otu")
                nc.vector.tensor_copy(otu[:], po[s][:, :])
                zs = nrm_pool.tile([128, 4], F32, name=f"zs_{hp}_{ib}_{s}", tag="zs")
                nc.gpsimd.dma_start(zs[:], otu[64:65, :])
                zr = nrm_pool.tile([128, 4], F32, name=f"zr_{hp}_{ib}_{s}", tag="zr")
                nc.vector.reciprocal(zr[:], zs[:])
                zinv = nrm_pool.tile([1, 512], F32, name=f"zinv_{hp}_{ib}_{s}", tag="zinv")
                nc.gpsimd.dma_start(zinv[:], zr[:])
                zb = nrm_pool.tile([64, 512], F32, name=f"zb_{hp}_{ib}_{s}", tag="zb")
                nc.gpsimd.partition_broadcast(zb[:], zinv[:])
                nc.vector.tensor_tensor(
                    ATn[hp][64 * s : 64 * s + 64, ib * 512 : (ib + 1) * 512],
                    otu[0:64, :],
                    zb[:],
                    op=ALU.mult,
                )

        def proj(tp):
            y_sb = y_pool.tile([128, C], F32, name=f"y_sb_{tp}", tag="y_sb")
            for oc in range(2):
                py = ps_acc.tile([128, 384], F32, name=f"ps_y_{tp}_{oc}", tag="acc")
                for cp in range(CC):
                    nc.tensor.matmul(
                        py[:],
                        ATn[cp][:, tp * 128 : (tp + 1) * 128],
                        Wp[cp][:, oc * 384 : (oc + 1) * 384],
                        start=(cp == 0),
                        stop=False,
                    )
                nc.tensor.matmul(
                    py[:],
                    ones_row[:, 0:128],
                    bp_sb[:, oc * 384 : (oc + 1) * 384],
                    start=False,
                    stop=True,
                )
                nc.vector.tensor_copy(y_sb[:, oc * 384 : (oc + 1) * 384], py[:])
            nc.sync.dma_start(y_d[tp * 128 : (tp + 1) * 128, :], y_sb[:])

        # Emission schedule: attention(hp, ib=0) needs Vp[0..3] + qkT pair hp;
        # start attention as early as possible, ib-major so proj overlaps ib=1.
        for tp in range(4):
            v_chunk(tp)
        qk_pair(0)
        attention(0, 0)
        qk_pair(1)
        attention(1, 0)
        for tp in range(4, 8):
            v_chunk(tp)
        for hp in range(2, 6):
            qk_pair(hp)
            attention(hp, 0)
        phase_ctx.close()  # release xt/xsb/w SBUF (all readers emitted)
        attention(0, 1)
        attention(1, 1)
        for tp in range(4):
            proj(tp)
            if tp + 2 < 6:
                attention(tp + 2, 1)
        for tp in range(4, 8):
            proj(tp)


def kernel(x, W_attn, b_attn, W_proj, b_proj, _trace=False, _trace_kwargs=None):
    import ml_dtypes

    bf16 = ml_dtypes.bfloat16
    x = np.ascontiguousarray(np.asarray(x).astype(bf16))
    W_attn = np.ascontiguousarray(np.asarray(W_attn).astype(bf16))
    b_attn = np.ascontiguousarray(np.asarray(b_attn).astype(bf16)).reshape(1, 3 * C)
    W_proj = np.ascontiguousarray(np.asarray(W_proj).astype(bf16))
    b_proj = np.ascontiguousarray(np.asarray(b_proj).astype(bf16)).reshape(1, C)

    if "prog" not in _PROGRAM_CACHE:
        _PROGRAM_CACHE["prog"] = build_program()
    nc = _PROGRAM_CACHE["prog"]

    in_maps = [
        {
            "x": np.ascontiguousarray(x[b]),
            "W_attn": W_attn,
            "b_attn": b_attn,
            "W_proj": W_proj,
            "b_proj": b_proj,
        }
        for b in range(NCORES)
    ]
    res = run_bass_kernel_spmd(
        nc,
        in_maps,
        core_ids=list(range(NCORES)),
        trace=_trace,
        **(_trace_kwargs or {}),
    )
    out = np.stack([res.results[b]["y"] for b in range(NCORES)], axis=0)
    if _trace:
        return out, res
    return out


if __name__ == "__main__":
    rng = np.random.default_rng(0)
    x = rng.standard_normal((NCORES, T, C)).astype(np.float32)
    W_attn = (rng.standard_normal((C, 3 * C)) * 0.02).astype(np.float32)
    b_attn = np.zeros(3 * C, np.float32)
    W_proj = (rng.standard_normal((C, C)) * 0.02).astype(np.float32)
    b_proj = np.zeros(C, np.float32)
    y = kernel(x=x, W_attn=W_attn, b_attn=b_attn, W_proj=W_proj, b_proj=b_proj)
    print("out", y.shape, y.dtype, np.abs(y).max())


# revision 22
# speedup vs baseline: 1.1710x; 1.1710x over previous
"""Causal self-attention kernel for TRN2 (8 NeuronCores, Bass/Tile).

Problem: B=8, T=1024, C=768, H=12, HD=64.
  qkv = x @ W_attn + b_attn ; causal softmax attention ; y = att_out @ W_proj + b_proj

Sharding: pure data-parallel over batch — core b computes batch element b.

Per-core dataflow (all matmuls in fp32r = full-rate reduced-precision fp32):
  xT   [768,1024]  = PE-transpose of x                      (lhsT/rhs source)
  qkT  [1536,1024] = (W_qk)^T-style projection: qkT[c',t] = sum_c W[c,c'] xT[c,t]
  V    [1024,768]  : V[t,c'] = sum_c xT[c,t] W_v[c,c']      (per-head Vp tiles with
                     a leading ones column -> PV matmul also produces Z row)
  per head h, i-block (512 cols):
     ST[j,i] = kT^T q  (K=64, causal-trimmed)   -> exp(0.125*ST) on ScalarE -> fp32r
     tri-mask on diagonal 128x128 sub-block (multiplicative, post-exp)
     OT'[0,:] = Z, OT'[1:65,:] = unnormalized attention out (transposed), accumulated
     ATn[c,t] = OT'[1:65]/Z  (DVE mult by gpsimd-broadcast 1/Z)
  y[t,:] = ATn^T-contraction with W_proj + b_proj
"""

import numpy as np

import concourse.bass as bass
import concourse.mybir as mybir
import concourse.tile as tile
from concourse import bacc
from concourse.bass_utils import run_bass_kernel_spmd

F32 = mybir.dt.float32
F32R = mybir.dt.float32r
BF16 = mybir.dt.bfloat16
AF = mybir.ActivationFunctionType
ALU = mybir.AluOpType

T, C, H, HD = 1024, 768, 12, 64
NCORES = 8
CC = C // 128          # 6 contraction chunks
TP = T // 128          # 8 t-chunks of 128
TB = T // 512          # 2 t-blocks of 512
QKCP = 2 * C // 128    # 12 qkT partition tiles
SCALE = 1.0 / 8.0      # 1/sqrt(64)

_PROGRAM_CACHE = {}


def build_program():
    nc = bacc.Bacc("TRN2", target_bir_lowering=False, debug=False)

    x_d = nc.dram_tensor("x", [T, C], BF16, kind="ExternalInput").ap()
    wa_d = nc.dram_tensor("W_attn", [C, 3 * C], BF16, kind="ExternalInput").ap()
    ba_d = nc.dram_tensor("b_attn", [1, 3 * C], BF16, kind="ExternalInput").ap()
    wp_d = nc.dram_tensor("W_proj", [C, C], BF16, kind="ExternalInput").ap()
    bp_d = nc.dram_tensor("b_proj", [1, C], BF16, kind="ExternalInput").ap()
    y_d = nc.dram_tensor("y", [T, C], F32, kind="ExternalOutput").ap()

    with tile.TileContext(nc) as tc:
        _emit(nc, tc, x_d, wa_d, ba_d, wp_d, bp_d, y_d)
    nc.compile()
    return nc


def _emit(nc, tc, x_d, wa_d, ba_d, wp_d, bp_d, y_d):
    from contextlib import ExitStack

    ctx = ExitStack()
    with ctx:
        const_pool = ctx.enter_context(tc.tile_pool(name="consts", bufs=1))
        # ps_work holds the merged [128,1024] ST tiles (2 banks each);
        # ps_acc holds 1-bank accumulation tiles (qk/v/y/ot').
        ps_work = ctx.enter_context(tc.tile_pool(name="ps_work", bufs=2, space="PSUM"))
        ps_acc = ctx.enter_context(tc.tile_pool(name="ps_acc", bufs=2, space="PSUM"))

        # ---- constants -------------------------------------------------
        ident_f32 = const_pool.tile([128, 128], F32, name="ident_f32")
        nc.gpsimd.memset(ident_f32[:], 0.0)
        nc.gpsimd.affine_select(
            out=ident_f32[:], in_=ident_f32[:], compare_op=ALU.not_equal, fill=1.0,
            base=0, pattern=[[-1, 128]], channel_multiplier=1,
        )
        ident = const_pool.tile([128, 128], BF16, name="ident")
        nc.vector.tensor_copy(ident[:], ident_f32[:])
        # tri[j, i] = 1.0 if j <= i else 0.0   (keep lower-causal in [j,i] layout)
        tri_f32 = const_pool.tile([128, 128], F32, name="tri_f32")
        nc.gpsimd.memset(tri_f32[:], 1.0)
        nc.gpsimd.affine_select(
            out=tri_f32[:], in_=tri_f32[:], compare_op=ALU.is_ge, fill=0.0,
            base=0, pattern=[[1, 128]], channel_multiplier=-1,
        )
        tri = const_pool.tile([128, 128], BF16, name="tri")
        nc.vector.tensor_copy(tri[:], tri_f32[:])
        ones32 = const_pool.tile([128, 16], F32, name="ones32")
        nc.gpsimd.memset(ones32[:], 1.0)
        ones_row = const_pool.tile([1, 512], BF16, name="ones_row")
        nc.gpsimd.memset(ones_row[:], 1.0)

        # warm the exp table set early (hidden under input DMA)
        expwarm = const_pool.tile([1, 1], F32, name="expwarm")
        nc.scalar.activation(expwarm[:], ones_row[0:1, 0:1], AF.Exp)

        # ---- phase A: load x (needed first), build xT [768, 1024] ------
        phase_ctx = ExitStack()
        xt_pool = phase_ctx.enter_context(tc.tile_pool(name="xt", bufs=1, side="right"))
        xsb_pool = phase_ctx.enter_context(tc.tile_pool(name="xsb", bufs=1, side="right"))

        xT = []
        for cc in range(CC):
            t_ = xt_pool.tile([128, T], BF16, name=f"xT_{cc}", tag=f"xT{cc}")
            xT.append(t_)
        # all 8 x chunks resident (bufs=8): the DMAs issue back-to-back with
        # no slot-reuse waits, so the W DMAs behind them on the queue are
        # never head-of-line blocked.
        xsb = []
        for tp in range(TP):
            x_sb = xsb_pool.tile([128, C], BF16, name=f"x_sb_{tp}", tag=f"x_sb{tp}")
            nc.sync.dma_start(x_sb[:], x_d[tp * 128 : (tp + 1) * 128, :])
            xsb.append(x_sb)

        # W_attn: 3 DMAs per row-chunk, V columns first, then q then k
        w_pool = phase_ctx.enter_context(tc.tile_pool(name="w", bufs=1, side="right"))
        W = []
        for cc in range(CC):
            w_t = w_pool.tile([128, 3 * C], BF16, name=f"W_{cc}", tag=f"W{cc}")
            W.append(w_t)
        for part in (2, 0, 1):
            for cc in range(CC):
                nc.sync.dma_start(
                    W[cc][:, part * C : (part + 1) * C],
                    wa_d[cc * 128 : (cc + 1) * 128, part * C : (part + 1) * C],
                )

        # bias loads go behind the big DMAs (first needed ~40us in; the
        # strided ba_col DMA has many tiny descriptors)
        ba_sb = const_pool.tile([1, 3 * C], BF16, name="ba_sb")
        nc.sync.dma_start(ba_sb[:], ba_d[:, :])
        bp_sb = const_pool.tile([1, C], BF16, name="bp_sb")
        nc.sync.dma_start(bp_sb[:], bp_d[:, :])
        # column layout of b_attn qk-part for per-partition bias add:
        # ba_col[p, cp] = b_attn[cp*128 + p]  (strided DMA, one-time, 3KB)
        ba_col = const_pool.tile([128, QKCP], F32, name="ba_col")
        ba_colb = const_pool.tile([128, QKCP], BF16, name="ba_colb")
        nc.sync.dma_start(
            ba_colb[:],
            ba_d[:, 0 : QKCP * 128].rearrange("a (cp p) -> (a p) cp", p=128),
        )
        nc.vector.tensor_copy(ba_col[:], ba_colb[:])

        for tp in range(TP):
            for cc in range(CC):
                pt = ps_work.tile([128, 128], BF16, name=f"ps_xt_{tp}_{cc}", tag="ps")
                nc.tensor.transpose(pt[:], xsb[tp][:, cc * 128 : (cc + 1) * 128], ident[:])
                nc.vector.tensor_copy(xT[cc][:, tp * 128 : (tp + 1) * 128], pt[:])

        # ---- phase B: Vp then qkT (attention needs all Vp) -------------
        vp_pool = ctx.enter_context(tc.tile_pool(name="vp", bufs=1))
        Vp = []
        for tp in range(TP):
            t_ = vp_pool.tile([128, H * 65], BF16, name=f"Vp_{tp}", tag=f"Vp{tp}")
            Vp.append(t_)
            nc.vector.tensor_copy(
                t_.rearrange("p (h e) -> p h e", e=65)[:, :, 64:65],
                ones32[:, 0:H].rearrange("p (h e) -> p h e", e=1),
            )

        def v_chunk(tp):
            for vc in range(2):  # v cols [1536+384*vc : 1536+384*(vc+1)]
                pv = ps_acc.tile([128, 384], F32, name=f"ps_v_{vc}_{tp}", tag="acc")
                for cc in range(CC):
                    nc.tensor.matmul(
                        pv[:],
                        xT[cc][:, tp * 128 : (tp + 1) * 128],
                        W[cc][:, 1536 + vc * 384 : 1536 + (vc + 1) * 384],
                        start=(cc == 0),
                        stop=False,
                    )
                nc.tensor.matmul(
                    pv[:],
                    ones_row[:, 0:128],
                    ba_sb[:, 1536 + vc * 384 : 1536 + (vc + 1) * 384],
                    start=False,
                    stop=True,
                )
                for hh in range(6):  # heads 6*vc + hh
                    h = 6 * vc + hh
                    nc.scalar.copy(
                        Vp[tp][:, h * 65 : h * 65 + 64],
                        pv[:, hh * 64 : (hh + 1) * 64],
                    )

        qkt_pool = ctx.enter_context(tc.tile_pool(name="qkt", bufs=1))
        qkT = []
        for cp in range(QKCP):
            t_ = qkt_pool.tile([128, T], BF16, name=f"qkT_{cp}", tag=f"qkT{cp}")
            qkT.append(t_)

        def qk_pair(hp):
            for tb in range(TB):
                for cp in (hp, 6 + hp):
                    pq = ps_acc.tile([128, 512], F32, name=f"ps_qk_{cp}_{tb}", tag="acc")
                    for cc in range(CC):
                        nc.tensor.matmul(
                            pq[:],
                            W[cc][:, cp * 128 : (cp + 1) * 128],
                            xT[cc][:, tb * 512 : (tb + 1) * 512],
                            start=(cc == 0),
                            stop=(cc == CC - 1),
                        )
                    # b_attn[c'] folded in as a per-partition scalar add
                    nc.vector.tensor_scalar_add(
                        qkT[cp][:, tb * 512 : (tb + 1) * 512],
                        pq[:],
                        ba_col[:, cp : cp + 1],
                    )

        # ---- W_proj prefetch ------------------------------------------
        wp_pool = ctx.enter_context(tc.tile_pool(name="wp", bufs=1))
        Wp = []
        for cc in range(CC):
            w_t = wp_pool.tile([128, C], BF16, name=f"Wp_{cc}", tag=f"Wp{cc}")
            nc.sync.dma_start(w_t[:], wp_d[cc * 128 : (cc + 1) * 128, :])
            Wp.append(w_t)

        # ---- phase C/D: attention (ib-major) interleaved with proj -----
        atn_pool = ctx.enter_context(tc.tile_pool(name="atn", bufs=1))
        ATn = []
        for cp in range(CC):
            t_ = atn_pool.tile([128, T], BF16, name=f"ATn_{cp}", tag=f"ATn{cp}")
            ATn.append(t_)

        est_pool = ctx.enter_context(tc.tile_pool(name="est", bufs=8))
        nrm_pool = ctx.enter_context(tc.tile_pool(name="nrm", bufs=3))
        y_pool = ctx.enter_context(tc.tile_pool(name="ysb", bufs=2))

        def attention(hp, ib):
            qt = qkT[hp]
            kt = qkT[6 + hp]
            po = {}
            for s in range(2):  # head 2*hp + s
                po[s] = ps_acc.tile([65, 512], F32, name=f"ps_ot_{hp}_{ib}_{s}", tag="ot", bufs=2)
            njc = 4 * (ib + 1)
            for jc in range(njc):
                r = jc - 4 * ib
                col0 = max(r, 0) * 128
                # merged pair tile: head A in cols [0:512], head B in [512:1024]
                pst = ps_work.tile([128, 1024], F32, name=f"ps_st_{hp}_{ib}_{jc}", tag="ps")
                for s in range(2):
                    r0 = 64 * s
                    # row-packed pair: s=0 uses PE rows 0-63, s=1 rows 64-127
                    nc.tensor.matmul(
                        pst[:, 512 * s + col0 : 512 * s + 512],
                        kt[r0 : r0 + 64, jc * 128 : (jc + 1) * 128],
                        qt[r0 : r0 + 64, ib * 512 + col0 : (ib + 1) * 512],
                        start=True,
                        stop=True,
                    )
                est = est_pool.tile([128, 1024], BF16, name=f"est_{hp}_{ib}_{jc}", tag="est")
                nw = 512 - col0
                nc.scalar.activation(
                    est.rearrange("p (a f) -> p a f", a=2)[:, :, col0:512],
                    pst.rearrange("p (a f) -> p a f", a=2)[:, :, col0:512],
                    AF.Exp,
                    scale=SCALE,
                )
                if r >= 0:
                    for s in range(2):
                        # mask the diagonal 128x128 sub-block (multiplicative)
                        nc.vector.tensor_tensor(
                            est[:, 512 * s + col0 : 512 * s + col0 + 128],
                            est[:, 512 * s + col0 : 512 * s + col0 + 128],
                            tri[:],
                            op=ALU.mult,
                        )
                for s in range(2):
                    h = 2 * hp + s
                    nc.tensor.matmul(
                        po[s][:, col0:512],
                        Vp[jc][:, h * 65 : h * 65 + 65],
                        est[:, 512 * s + col0 : 512 * s + 512],
                        start=(jc == 0),
                        stop=(jc == njc - 1),
                    )
            # normalization: ATn rows = OT'[0:64] / Z  (Z = row 64).
            # Copy OT' to SBUF right away so the PSUM slot frees in ~0.7us;
            # the (long-latency, off-critical-path) normalization then runs
            # entirely from SBUF. [1,512] DVE reciprocal is ~3.4us (single
            # partition, 8 cyc/elem): scatter Z across 128 partitions via
            # DMA, reciprocal at [128,4], gather back.
            for s in range(2):
                otu = nrm_pool.tile([65, 512], F32, name=f"otu_{hp}_{ib}_{s}", tag="otu")
                nc.vector.tensor_copy(otu[:], po[s][:, :])
                zs = nrm_pool.tile([128, 4], F32, name=f"zs_{hp}_{ib}_{s}", tag="zs")
                nc.gpsimd.dma_start(zs[:], otu[64:65, :])
                zr = nrm_pool.tile([128, 4], F32, name=f"zr_{hp}_{ib}_{s}", tag="zr")
                nc.vector.reciprocal(zr[:], zs[:])
                zinv = nrm_pool.tile([1, 512], F32, name=f"zinv_{hp}_{ib}_{s}", tag="zinv")
                nc.gpsimd.dma_start(zinv[:], zr[:])
                zb = nrm_pool.tile([64, 512], F32, name=f"zb_{hp}_{ib}_{s}", tag="zb")
                nc.gpsimd.partition_broadcast(zb[:], zinv[:])
                nc.vector.tensor_tensor(
                    ATn[hp][64 * s : 64 * s + 64, ib * 512 : (ib + 1) * 512],
                    otu[0:64, :],
                    zb[:],
                    op=ALU.mult,
                )

        def proj(tp):
            y_sb = y_pool.tile([128, C], F32, name=f"y_sb_{tp}", tag="y_sb")
            for oc in range(2):
                py = ps_acc.tile([128, 384], F32, name=f"ps_y_{tp}_{oc}", tag="acc")
                for cp in range(CC):
                    nc.tensor.matmul(
                        py[:],
                        ATn[cp][:, tp * 128 : (tp + 1) * 128],
                        Wp[cp][:, oc * 384 : (oc + 1) * 384],
                        start=(cp == 0),
                        stop=False,
                    )
                nc.tensor.matmul(
                    py[:],
                    ones_row[:, 0:128],
                    bp_sb[:, oc * 384 : (oc + 1) * 384],
                    start=False,
                    stop=True,
                )
                nc.vector.tensor_copy(y_sb[:, oc * 384 : (oc + 1) * 384], py[:])
            nc.sync.dma_start(y_d[tp * 128 : (tp + 1) * 128, :], y_sb[:])

        # Emission schedule: attention(hp, ib=0) needs Vp[0..3] + qkT pair hp;
        # start attention as early as possible, ib-major so proj overlaps ib=1.
        for tp in range(4):
            v_chunk(tp)
        qk_pair(0)
        attention(0, 0)
        qk_pair(1)
        attention(1, 0)
        for tp in range(4, 8):
            v_chunk(tp)
        for hp in range(2, 6):
            qk_pair(hp)
            attention(hp, 0)
        phase_ctx.close()  # release xt/xsb/w SBUF (all readers emitted)
        attention(0, 1)
        attention(1, 1)
        proj(0)
        attention(2, 1)
        proj(1)
        attention(3, 1)
        proj(2)
        attention(4, 1)
        proj(3)
        attention(5, 1)
        for tp in range(4, 8):
            proj(tp)


def kernel(x, W_attn, b_attn, W_proj, b_proj, _trace=False, _trace_kwargs=None):
    import ml_dtypes

    bf16 = ml_dtypes.bfloat16
    x = np.ascontiguousarray(np.asarray(x).astype(bf16))
    W_attn = np.ascontiguousarray(np.asarray(W_attn).astype(bf16))
    b_attn = np.ascontiguousarray(np.asarray(b_attn).astype(bf16)).reshape(1, 3 * C)
    W_proj = np.ascontiguousarray(np.asarray(W_proj).astype(bf16))
    b_proj = np.ascontiguousarray(np.asarray(b_proj).astype(bf16)).reshape(1, C)

    if "prog" not in _PROGRAM_CACHE:
        _PROGRAM_CACHE["prog"] = build_program()
    nc = _PROGRAM_CACHE["prog"]

    in_maps = [
        {
            "x": np.ascontiguousarray(x[b]),
            "W_attn": W_attn,
            "b_attn": b_attn,
            "W_proj": W_proj,
            "b_proj": b_proj,
        }
        for b in range(NCORES)
    ]
    res = run_bass_kernel_spmd(
        nc,
        in_maps,
        core_ids=list(range(NCORES)),
        trace=_trace,
        **(_trace_kwargs or {}),
    )
    out = np.stack([res.results[b]["y"] for b in range(NCORES)], axis=0)
    if _trace:
        return out, res
    return out


if __name__ == "__main__":
    rng = np.random.default_rng(0)
    x = rng.standard_normal((NCORES, T, C)).astype(np.float32)
    W_attn = (rng.standard_normal((C, 3 * C)) * 0.02).astype(np.float32)
    b_attn = np.zeros(3 * C, np.float32)
    W_proj = (rng.standard_normal((C, C)) * 0.02).astype(np.float32)
    b_proj = np.zeros(C, np.float32)
    y = kernel(x=x, W_attn=W_attn, b_attn=b_attn, W_proj=W_proj, b_proj=b_proj)
    print("out", y.shape, y.dtype, np.abs(y).max())
